# revision 12
# baseline (speedup 1.0000x reference)
"""Mamba (4-layer selective-SSM) Bass kernel for Trainium2, 8 NeuronCores.

Sharding: pure data-parallel over batch (B=8 -> one batch element per core).
Each core runs the full model on its batch element; no collectives.

Model (hardcoded from the problem spec):
  B=8, L=512, IN_DIM=32, D_MODEL=512, N_LAYERS=4, D_INNER=1024,
  D_STATE=16, D_CONV=4, DT_RANK=32, OUT_DIM=1.

Per-core layout conventions:
  "D-layout": [d_p (tiles of 128 partitions), t in free dim (512)].
  h (residual stream) lives as [128, 4, 512] fp32: h[p, i, t] = H[t, 128*i+p].
  d_inner tensors live as [128, 8, 512]: x[p, g, t] = X[t, 128*g+p].

The selective scan runs per (d_inner-block g, d_state-group):
  a = exp(delta * A[d,n])   (ACT, per-partition scale)
  b = (delta*x) * B_t[n]    (DVE, B broadcast across partitions via DRAM bounce)
  h_t = a*h + b             (DVE tensor_tensor_scan along t)
  y  += sum_n h*C_t[n]      (DVE mult + tree reduce)
"""

import numpy as np
import ml_dtypes

B, L, IN_DIM, OUT_DIM = 8, 512, 32, 1
DM, NL = 512, 4
DI = 2 * DM          # 1024
N = 16               # d_state
KC = 4               # d_conv
DR = 32              # dt_rank
NDT = DM // 128      # 4 d_model tiles
NGT = DI // 128      # 8 d_inner tiles
EPS = 1e-5

_CACHE = {}


def _build():
    import concourse.mybir as mybir
    from concourse import bacc
    from concourse.tile import TileContext

    F32 = mybir.dt.float32
    BF = mybir.dt.bfloat16
    AF = mybir.ActivationFunctionType
    OP = mybir.AluOpType

    nc = bacc.Bacc(None, target_bir_lowering=False)

    # ---- DRAM parameters (per-core inputs; weights identical across cores) ----
    d_xT = nc.declare_dram_parameter("xT", [IN_DIM, L], F32, isOutput=False)
    d_enc_wT = nc.declare_dram_parameter("enc_wT", [IN_DIM, DM], F32, isOutput=False)
    d_enc_bc = nc.declare_dram_parameter("enc_bc", [128, NDT], F32, isOutput=False)
    d_w_inT = nc.declare_dram_parameter("w_inT", [NL, 128, NDT, 2 * DI], BF, isOutput=False)
    d_conv_w = nc.declare_dram_parameter("conv_wc", [NL, 128, NGT, KC], F32, isOutput=False)
    d_conv_b = nc.declare_dram_parameter("conv_bc", [NL, 128, NGT], F32, isOutput=False)
    d_xp_wT = nc.declare_dram_parameter("xp_wT", [NL, 128, NGT, DR + 2 * N], BF, isOutput=False)
    d_dtp_wT = nc.declare_dram_parameter("dtp_wT", [NL, DR, DI], BF, isOutput=False)
    d_dtp_bc = nc.declare_dram_parameter("dtp_bc", [NL, 128, NGT], F32, isOutput=False)
    d_A = nc.declare_dram_parameter("A_c", [NL, 128, NGT, N], F32, isOutput=False)
    d_D = nc.declare_dram_parameter("D_c", [NL, 128, NGT], F32, isOutput=False)
    d_out_wT = nc.declare_dram_parameter("out_wT", [NL, 128, NGT, DM], BF, isOutput=False)
    d_dec_w1T = nc.declare_dram_parameter("dec_w1T", [128, NDT, DM // 2], BF, isOutput=False)
    d_dec_b1c = nc.declare_dram_parameter("dec_b1c", [128, 2], F32, isOutput=False)
    d_dec_w2T = nc.declare_dram_parameter("dec_w2T", [128, 2, OUT_DIM], BF, isOutput=False)
    d_dec_b2 = nc.declare_dram_parameter("dec_b2", [1, 1], F32, isOutput=False)
    d_out = nc.declare_dram_parameter("out", [1, 1], F32, isOutput=True)

    with TileContext(nc) as tc:
        with tc.tile_pool(name="state", bufs=1) as sp, \
             tc.tile_pool(name="wp", bufs=1) as wp, \
             tc.tile_pool(name="scan", bufs=1) as scp, \
             tc.tile_pool(name="dram", bufs=1, space="DRAM") as dp, \
             tc.tile_pool(name="ps", bufs=3, space="PSUM") as pp, \
             tc.tile_pool(name="ps2", bufs=1, space="PSUM") as pp2:

            # ---------------- persistent tiles ----------------
            h = sp.tile([128, NDT, L], F32)           # residual stream
            normed = sp.tile([128, NDT, L], BF)
            xr_pre = sp.tile([128, NGT, L + KC - 1], BF)   # conv input, 3-col left pad
            conv_acc = sp.tile([128, NGT, L], BF)
            xr_silu = sp.tile([128, NGT, L], BF)
            zsil = sp.tile([128, NGT, L], BF)
            delta = sp.tile([128, NGT, L], F32)
            u = sp.tile([128, NGT, L], BF)
            y = sp.tile([128, NGT, L], F32)
            B_all = sp.tile([128, N, L], BF)
            C_all = sp.tile([128, N, L], BF)
            dt_rhs = sp.tile([DR, L], BF)
            bc_sb = sp.tile([2 * N, L], BF)
            ones_col = sp.tile([128, 1], BF)          # for partition-sum matmul
            ones_row = sp.tile([1, 128], F32)         # for partition-broadcast matmul
            s_row = sp.tile([1, L], F32)              # rsqrt(mean h^2)
            zsg = sp.tile([128, L], BF)               # sigmoid(z) scratch

            # per-layer weight tiles (reloaded each layer)
            w_in = wp.tile([128, NDT, 2 * DI], BF)
            conv_w = wp.tile([128, NGT, KC], F32)
            conv_b = wp.tile([128, NGT], F32)
            xp_w = wp.tile([128, NGT, DR + 2 * N], BF)
            dtp_w = wp.tile([DR, DI], BF)
            dtp_b = wp.tile([128, NGT], F32)
            A_t = wp.tile([128, NGT, N], F32)
            D_t = wp.tile([128, NGT], F32)
            out_w = wp.tile([128, NGT, DM], BF)

            bc_scr = dp.tile([2 * N, L], BF)          # DRAM bounce for B/C broadcast

            eps_col = sp.tile([1, 1], F32)
            nc.vector.memset(ones_col[:], 1.0)
            nc.vector.memset(ones_row[:], 1.0)
            nc.vector.memset(eps_col[:], EPS)
            nc.vector.memset(xr_pre[:, :, 0:KC - 1], 0.0)

            # ---------------- encoder: h = x @ enc_w.T + enc_b ----------------
            xT = sp.tile([IN_DIM, L], F32)
            enc_wT = sp.tile([IN_DIM, DM], F32)
            nc.sync.dma_start(xT[:], d_xT[:])
            nc.sync.dma_start(enc_wT[:], d_enc_wT[:])
            enc_bc = sp.tile([128, NDT], F32)
            nc.sync.dma_start(enc_bc[:], d_enc_bc[:])
            for i in range(NDT):
                mm = pp.tile([128, L], F32, tag="mm")
                nc.tensor.matmul(mm[:], enc_wT[:, i * 128:(i + 1) * 128], xT[:])
                nc.scalar.activation(h[:, i, :], mm[:], AF.Identity,
                                     bias=enc_bc[:, i:i + 1])

            # ---------------- layers ----------------
            for l in range(NL):
                # load weights for this layer
                nc.sync.dma_start(w_in[:], d_w_inT[l])
                nc.sync.dma_start(conv_w[:], d_conv_w[l])
                nc.sync.dma_start(conv_b[:], d_conv_b[l])
                nc.sync.dma_start(xp_w[:], d_xp_wT[l])
                nc.sync.dma_start(dtp_w[:], d_dtp_wT[l])
                nc.sync.dma_start(dtp_b[:], d_dtp_bc[l])
                nc.sync.dma_start(A_t[:], d_A[l])
                nc.sync.dma_start(D_t[:], d_D[l])
                nc.sync.dma_start(out_w[:], d_out_wT[l])

                # ---- RMSNorm: normed = h * rsqrt(mean_d h^2 + eps) ----
                # conv_acc doubles as h^2 scratch (disjoint lifetime)
                hsq = conv_acc[:, 0:NDT, :]
                nc.scalar.activation(hsq, h[:], AF.Square)
                m_ps = pp2.tile([1, L], F32, tag="m1")
                for i in range(NDT):
                    nc.tensor.matmul(m_ps[:], ones_col[:], hsq[:, i, :],
                                     start=(i == 0), stop=(i == NDT - 1))
                # s = exp(-0.5 * ln(m/DM + eps))
                nc.scalar.activation(s_row[:], m_ps[:], AF.Ln,
                                     bias=eps_col[:], scale=1.0 / DM)
                nc.scalar.activation(s_row[:], s_row[:], AF.Exp, scale=-0.5)
                sbc_ps = pp.tile([128, L], F32, tag="mm")
                nc.tensor.matmul(sbc_ps[:], ones_row[:], s_row[:])
                for i in range(NDT):
                    nc.vector.tensor_tensor(normed[:, i, :], h[:, i, :],
                                            sbc_ps[:], OP.mult)

                # ---- in_proj (x half) -> xr_pre ----
                for g in range(NGT):
                    mm = pp.tile([128, L], F32, tag="mm")
                    for i in range(NDT):
                        nc.tensor.matmul(
                            mm[:], w_in[:, i, g * 128:(g + 1) * 128],
                            normed[:, i, :],
                            start=(i == 0), stop=(i == NDT - 1))
                    nc.scalar.copy(xr_pre[:, g, KC - 1:], mm[:])

                # ---- causal depthwise conv ----
                for g in range(NGT):
                    nc.vector.tensor_scalar(
                        conv_acc[:, g, :], xr_pre[:, g, 0:L],
                        conv_w[:, g, 0:1], conv_b[:, g:g + 1],
                        OP.mult, OP.add)
                    for k in range(1, KC):
                        nc.vector.scalar_tensor_tensor(
                            conv_acc[:, g, :], xr_pre[:, g, k:k + L],
                            conv_w[:, g, k:k + 1], conv_acc[:, g, :],
                            OP.mult, OP.add)
                # silu(x) = x * sigmoid(x); sigmoid scratch reuses xr_pre body
                sg = xr_pre[:, :, KC - 1:]
                nc.scalar.activation(sg, conv_acc[:], AF.Sigmoid)
                nc.vector.tensor_tensor(xr_silu[:], conv_acc[:], sg, OP.mult)

                # ---- in_proj (z half) + silu, deferred-fused here ----
                for g in range(NGT):
                    mm = pp.tile([128, L], F32, tag="mm")
                    for i in range(NDT):
                        nc.tensor.matmul(
                            mm[:], w_in[:, i, DI + g * 128:DI + (g + 1) * 128],
                            normed[:, i, :],
                            start=(i == 0), stop=(i == NDT - 1))
                    nc.scalar.activation(zsg[:], mm[:], AF.Sigmoid)
                    nc.vector.tensor_tensor(zsil[:, g, :], zsg[:], mm[:],
                                            OP.mult)

                # ---- x_proj: dbc = xr_silu @ xp_w.T ----
                dbc_ps = pp2.tile([DR + 2 * N, L], F32, tag="dbc")
                for g in range(NGT):
                    nc.tensor.matmul(dbc_ps[:], xp_w[:, g, :], xr_silu[:, g, :],
                                     start=(g == 0), stop=(g == NGT - 1))
                nc.scalar.copy(dt_rhs[:], dbc_ps[0:DR, :])
                nc.scalar.copy(bc_sb[:], dbc_ps[DR:DR + 2 * N, :])

                # broadcast B,C rows across partitions via DRAM bounce
                nc.sync.dma_start(bc_scr[:], bc_sb[:])
                bsrc = bc_scr[0:N, :].rearrange("(o n) t -> o n t", o=1) \
                                     .broadcast_to([128, N, L])
                csrc = bc_scr[N:2 * N, :].rearrange("(o n) t -> o n t", o=1) \
                                         .broadcast_to([128, N, L])
                nc.sync.dma_start(B_all[:], bsrc)
                nc.sync.dma_start(C_all[:], csrc)

                # ---- dt_proj + softplus -> delta ----
                for g in range(NGT):
                    mm = pp.tile([128, L], F32, tag="mm")
                    nc.tensor.matmul(mm[:], dtp_w[:, g * 128:(g + 1) * 128],
                                     dt_rhs[:])
                    et = pp.tile([128, L], F32, tag="mm")
                    # e = exp(raw + dt_bias); delta = ln(1 + e)
                    nc.scalar.activation(et[:], mm[:], AF.Exp,
                                         bias=dtp_b[:, g:g + 1])
                    nc.scalar.activation(delta[:, g, :], et[:], AF.Ln, bias=1.0)

                # ---- u = delta * xr_silu ; y = D * xr_silu ----
                nc.vector.tensor_tensor(u[:], delta[:], xr_silu[:], OP.mult)
                for g in range(NGT):
                    nc.vector.tensor_scalar(y[:, g, :], xr_silu[:, g, :],
                                            D_t[:, g:g + 1], None, OP.mult)

                # ---- selective scan ----
                for g in range(NGT):
                    for ng in range(2):
                        n0 = ng * 8
                        a_grp = scp.tile([128, 8, L], BF, tag="a", bufs=2)
                        b_grp = scp.tile([128, 8, L], BF, tag="b")
                        hs_grp = scp.tile([128, 8, L], BF, tag="hs")
                        tmp_grp = scp.tile([128, 8, L], BF, tag="tmp")
                        for k in range(8):
                            nc.scalar.activation(
                                a_grp[:, k, :], delta[:, g, :], AF.Exp,
                                scale=A_t[:, g, n0 + k:n0 + k + 1])
                        ub = u[:, g, :].rearrange("p (o t) -> p o t", o=1) \
                                       .broadcast_to([128, 8, L])
                        nc.vector.tensor_tensor(b_grp[:], ub,
                                                B_all[:, n0:n0 + 8, :], OP.mult)
                        for k in range(8):
                            nc.vector.tensor_tensor_scan(
                                hs_grp[:, k, :], a_grp[:, k, :], b_grp[:, k, :],
                                0.0, OP.mult, OP.add)
                        nc.vector.tensor_tensor(tmp_grp[:], hs_grp[:],
                                                C_all[:, n0:n0 + 8, :], OP.mult)
                        for half in (4, 2):
                            nc.vector.tensor_tensor(
                                tmp_grp[:, 0:half, :], tmp_grp[:, 0:half, :],
                                tmp_grp[:, half:2 * half, :], OP.add)
                        nc.vector.tensor_tensor(
                            tmp_grp[:, 0, :], tmp_grp[:, 0, :],
                            tmp_grp[:, 1, :], OP.add)
                        nc.vector.tensor_tensor(y[:, g, :], y[:, g, :],
                                                tmp_grp[:, 0, :], OP.add)

                # ---- gate: y2 = y * silu(z) ----
                y2 = sp.tile([128, NGT, L], BF, name=f"y2_{l}", tag="y2")
                nc.vector.tensor_tensor(y2[:], y[:], zsil[:], OP.mult)

                # ---- out_proj + residual ----
                for i in range(NDT):
                    mm = pp.tile([128, L], F32, tag="mm")
                    for g in range(NGT):
                        nc.tensor.matmul(mm[:], out_w[:, g, i * 128:(i + 1) * 128],
                                         y2[:, g, :],
                                         start=(g == 0), stop=(g == NGT - 1))
                    nc.vector.tensor_tensor(h[:, i, :], h[:, i, :], mm[:], OP.add)

            # ---------------- decoder head on last token ----------------
            dec_w1 = sp.tile([128, NDT, DM // 2], BF)
            dec_b1c = sp.tile([128, 2], F32)
            dec_w2 = sp.tile([128, 2, OUT_DIM], BF)
            dec_b2 = sp.tile([1, 1], F32)
            nc.sync.dma_start(dec_w1[:], d_dec_w1T[:])
            nc.sync.dma_start(dec_b1c[:], d_dec_b1c[:])
            nc.sync.dma_start(dec_w2[:], d_dec_w2T[:])
            nc.sync.dma_start(dec_b2[:], d_dec_b2[:])

            hlast = sp.tile([128, NDT], BF)
            nc.scalar.copy(hlast[:], h[:, :, L - 1])
            r1 = sp.tile([128, 2], BF)
            for j in range(2):
                mm1 = pp.tile([128, 1], F32, tag="mm")
                for i in range(NDT):
                    nc.tensor.matmul(mm1[:], dec_w1[:, i, j * 128:(j + 1) * 128],
                                     hlast[:, i:i + 1],
                                     start=(i == 0), stop=(i == NDT - 1))
                nc.scalar.activation(r1[:, j:j + 1], mm1[:], AF.Relu,
                                     bias=dec_b1c[:, j:j + 1])
            mm2 = pp.tile([1, 1], F32, tag="mm")
            for j in range(2):
                nc.tensor.matmul(mm2[:], dec_w2[:, j, :], r1[:, j:j + 1],
                                 start=(j == 0), stop=(j == 1))
            out_sb = sp.tile([1, 1], F32)
            nc.scalar.activation(out_sb[:], mm2[:], AF.Identity,
                                 bias=dec_b2[0:1, 0:1])
            nc.sync.dma_start(d_out[:], out_sb[:])

    nc.finalize()
    return nc


def prep_weights(inputs):
    """Host-side packing/transposition of all weights into kernel layouts."""
    bf16 = ml_dtypes.bfloat16
    f32 = np.float32

    def col(a, nt):  # (nt*128,) -> (128, nt)
        return np.ascontiguousarray(a.reshape(nt, 128).T).astype(f32)

    enc_wT = np.ascontiguousarray(inputs["enc_w"].T).astype(f32)       # (32,512)
    enc_bc = col(inputs["enc_b"], NDT)

    w_inT = np.empty((NL, 128, NDT, 2 * DI), dtype=bf16)
    conv_wc = np.empty((NL, 128, NGT, KC), dtype=f32)
    conv_bc = np.empty((NL, 128, NGT), dtype=f32)
    xp_wT = np.empty((NL, 128, NGT, DR + 2 * N), dtype=bf16)
    dtp_wT = np.empty((NL, DR, DI), dtype=bf16)
    dtp_bc = np.empty((NL, 128, NGT), dtype=f32)
    A_c = np.empty((NL, 128, NGT, N), dtype=f32)
    D_c = np.empty((NL, 128, NGT), dtype=f32)
    out_wT = np.empty((NL, 128, NGT, DM), dtype=bf16)

    for l in range(NL):
        W = inputs["in_proj_w"][l] * inputs["norm_w"][l][None, :]      # (2DI, DM)
        w_inT[l] = W.T.reshape(NDT, 128, 2 * DI).transpose(1, 0, 2).astype(bf16)
        conv_wc[l] = inputs["conv_w"][l, :, 0, :].reshape(NGT, 128, KC) \
                                                 .transpose(1, 0, 2).astype(f32)
        conv_bc[l] = inputs["conv_b"][l].reshape(NGT, 128).T.astype(f32)
        xp_wT[l] = inputs["x_proj_w"][l].T.reshape(NGT, 128, DR + 2 * N) \
                                          .transpose(1, 0, 2).astype(bf16)
        dtp_wT[l] = inputs["dt_proj_w"][l].T.astype(bf16)              # (32,1024)
        dtp_bc[l] = inputs["dt_proj_b"][l].reshape(NGT, 128).T.astype(f32)
        A_c[l] = (-np.exp(inputs["A_log"][l])).reshape(NGT, 128, N) \
                                              .transpose(1, 0, 2).astype(f32)
        D_c[l] = inputs["D"][l].reshape(NGT, 128).T.astype(f32)
        out_wT[l] = inputs["out_proj_w"][l].T.reshape(NGT, 128, DM) \
                                             .transpose(1, 0, 2).astype(bf16)

    dec_w1T = inputs["dec_w1"].T.reshape(NDT, 128, DM // 2) \
                                .transpose(1, 0, 2).astype(bf16)
    dec_b1c = inputs["dec_b1"].reshape(2, 128).T.astype(f32)
    dec_w2T = inputs["dec_w2"].T.reshape(2, 128, OUT_DIM) \
                                .transpose(1, 0, 2).astype(bf16)
    dec_b2 = inputs["dec_b2"].reshape(1, 1).astype(f32)

    return {
        "enc_wT": enc_wT, "enc_bc": enc_bc, "w_inT": w_inT,
        "conv_wc": conv_wc, "conv_bc": conv_bc, "xp_wT": xp_wT,
        "dtp_wT": dtp_wT, "dtp_bc": dtp_bc, "A_c": A_c, "D_c": D_c,
        "out_wT": out_wT, "dec_w1T": dec_w1T, "dec_b1c": dec_b1c,
        "dec_w2T": dec_w2T, "dec_b2": dec_b2,
    }


def get_nc():
    if "nc" not in _CACHE:
        _CACHE["nc"] = _build()
    return _CACHE["nc"]


def kernel(**inputs):
    from concourse.bass_utils import run_bass_kernel_spmd

    nc = get_nc()
    w = prep_weights(inputs)
    x = np.asarray(inputs["x"], dtype=np.float32)
    in_maps = []
    for c in range(B):
        m = dict(w)
        m["xT"] = np.ascontiguousarray(x[c].T)        # (32, 512)
        in_maps.append(m)
    res = run_bass_kernel_spmd(nc, in_maps, core_ids=list(range(B)))
    out = np.array([res.results[c]["out"][0, 0] for c in range(B)],
                   dtype=np.float32)
    return out
